# revision 28
# baseline (speedup 1.0000x reference)
"""BiGraphConv (GNN message passing) Trainium2 kernel, 8-core SPMD.

out = x_dst @ W_self.T + b_self + scatter_add_dst(w_e * x_src[src_e]) @ W_nei.T

Formulated aggregate-first, per dst-shard:
    agg[d]  = sum_{e: dst_e=d} w_e * x_src[src_e]     (gather + one-hot matmul)
    out'[d] = W_nei @ agg[d] + W_self @ x_dst[d] + b  (feature-major matmuls)

Sharding: dst nodes partitioned across 8 cores (12500 each); x_src replicated;
edges bucketed by (dst-core, dst-group, src-chunk) on host. Gathered x rows in
fp8 e3m4 (64B descriptors); one-hot construction in fp16 with narrowed
sub-windows per column rank (col0 covers group-local dst [0,48), col-last
[22,70), middles full 70) to cut DVE work; fp32 PSUM accumulate + transform.
Output assembled/transposed on host.
"""
import sys
import inspect
import re
import numpy as np

for _p in ("/opt/trn_rl_repo", "/root/.axon_site/_ro/trn_rl_repo"):
    if _p not in sys.path:
        sys.path.insert(0, _p)

from contextlib import ExitStack

import ml_dtypes
import concourse.bass as bass
import concourse.tile as tile
from concourse import bacc, mybir
from concourse.bass_utils import run_bass_kernel_spmd

# problem constants (hardcoded per task contract)
N_SRC = 100000
N_DST = 100000
E = 1250000
F = 64          # feature dim (in == out == 64)
NC = 8          # cores
SHARD = N_DST // NC          # 12500 dst rows per core
G = 70                       # dst rows per aggregation group
NG = (SHARD + G - 1) // G    # 179 groups per core
NCH = 4                      # src chunks (int16 index limit)
CHROWS = N_SRC // NCH        # 25000 rows per chunk window
W = 127                      # gather window width in 128-edge columns
                             # (ring limit: W*128/16+1 <= 1024 descriptors)
KB = 16                      # one-hot batch width in columns
DMA_SCRATCH = 16384          # SWDGE ring bytes per partition (default)
TCH = 490                    # transform chunk (dst cols; multiple of G)
NTC = (SHARD + TCH - 1) // TCH   # 26 transform chunks
USE_FP8 = True               # fp8 e3m4 gathered x rows + fp8 x_dst

# narrowed one-hot sub-windows (group-local dst ranges per column rank)
WLO = 48                     # col0 window [0, WLO)
HI0 = G - WLO                # col-last window [HI0, G) = [22, 70)

P = 128
XPAD = 128                   # padded fp16 row length (256B stride)
XPAD8 = 256                  # padded fp8 row length (256B stride)

_patched_gather = None


def _get_patched_gather(nc):
    """dma_gather with the 256B-payload assert relaxed for non-transpose.

    The ucode's row-stride field is in 256B units (elem_step stays 256B via
    the padded source), but the payload may be 64/128B; verified on HW.
    """
    global _patched_gather
    if _patched_gather is not None:
        return _patched_gather
    cls = type(nc.gpsimd)
    src = inspect.getsource(cls.dma_gather)
    src = src.replace(
        """        assert (
            elem_size_bytes > 0 and elem_size_bytes % 256 == 0
        )  # transpose restriction""",
        """        assert elem_size_bytes > 0
        if transpose:
            assert elem_size_bytes % 256 == 0""")
    src = re.sub(r"^    def dma_gather", "def dma_gather", src)
    src = re.sub(r"\n    ", "\n", src)
    ns = vars(sys.modules[cls.__module__]).copy()
    exec(compile(src, "<patched_dma_gather>", "exec"), ns)
    _patched_gather = ns["dma_gather"]
    return _patched_gather


def _host_prep(x_src, x_dst, edge_index_sd, edge_weight, W_nei, W_self, b_self):
    src = np.asarray(edge_index_sd[0], dtype=np.int64)
    dst = np.asarray(edge_index_sd[1], dtype=np.int64)
    ew = np.asarray(edge_weight, dtype=np.float32)
    x_dst = np.asarray(x_dst, dtype=np.float32)

    core = dst // SHARD
    dl = dst % SHARD          # shard-local dst id
    grp = dl // G
    gl = dl % G               # group-local dst id
    chunk = src // CHROWS

    order = np.lexsort((gl, chunk, grp, core))
    core_s = core[order]
    chunk_s = chunk[order]
    grp_s = grp[order]
    gl_s = gl[order]
    src_s = src[order]
    ew_s = ew[order]

    # per (core, group, chunk) slot counts + sub-window counts
    key = (core_s * NG + grp_s) * NCH + chunk_s
    NSLOT = NC * NG * NCH
    cnt = np.bincount(key, minlength=NSLOT).reshape(NC, NG, NCH)
    nb_lo = np.bincount(key[gl_s < HI0], minlength=NSLOT).reshape(
        NC, NG, NCH)                       # edges that MUST go to col0
    nb_hi = np.bincount(key[gl_s >= WLO], minlength=NSLOT).reshape(
        NC, NG, NCH)                       # edges that MUST go to col-last

    # shared structure: columns per slot = max over cores
    k_slot = (-(-cnt // P)).max(axis=0)    # [NG, NCH]
    # per-core assignment: top t -> col-last, bottom b -> col0, rest middles
    t_c = np.minimum(P, cnt - nb_lo)       # col-last load (all gl >= HI0..)
    t_c = np.maximum(t_c, 0)
    b_c = np.minimum(np.minimum(P, cnt - nb_hi), cnt - t_c)
    b_c = np.maximum(b_c, 0)
    m_c = cnt - t_c - b_c
    # fallback: full-width columns when narrowed windows infeasible anywhere
    full_slot = (k_slot == 1) | (m_c > P * np.maximum(k_slot - 2, 0)).any(
        axis=0)
    full_slot &= k_slot > 0

    # column inventory per chunk, group-ascending (= gather/consumption order)
    # within a slot: [col0(lo), middles(full), col-last(hi)] or all-full
    ncols_slot = k_slot                      # [NG, NCH]
    cidx_base = np.zeros((NG, NCH), dtype=np.int64)
    for r in range(NCH):
        np.cumsum(ncols_slot[:-1, r], out=cidx_base[1:, r])
    ncols_r = ncols_slot.sum(axis=0)         # gather cols per chunk

    # pm stream indices
    is_lo = (~full_slot) & (k_slot >= 2)     # slots contributing lo+hi cols
    lo_idx = np.zeros((NG, NCH), dtype=np.int64)
    nfull_slot = np.where(full_slot, k_slot, np.maximum(k_slot - 2, 0))
    full_base = np.zeros((NG, NCH), dtype=np.int64)
    nlo_r = np.zeros(NCH, dtype=np.int64)
    nfull_r = np.zeros(NCH, dtype=np.int64)
    for r in range(NCH):
        lo_idx[:, r] = np.cumsum(is_lo[:, r]) - is_lo[:, r]
        nlo_r[r] = is_lo[:, r].sum()
        full_base[:, r] = np.cumsum(nfull_slot[:, r]) - nfull_slot[:, r]
        nfull_r[r] = nfull_slot[:, r].sum()

    def pad16(n):
        return -(-int(n) // KB) * KB

    nlo_r_pad = np.array([pad16(n) for n in nlo_r])
    nfull_r_pad = np.array([pad16(n) for n in nfull_r])
    # pm table layout: [lo r0..r3 | hi r0..r3 | full r0..r3]
    lo_base = np.zeros(NCH, dtype=np.int64)
    np.cumsum(nlo_r_pad[:-1], out=lo_base[1:])
    hi_base = np.zeros(NCH, dtype=np.int64)
    np.cumsum(nlo_r_pad[:-1], out=hi_base[1:])
    hi_base += nlo_r_pad.sum()
    fu_base = np.zeros(NCH, dtype=np.int64)
    np.cumsum(nfull_r_pad[:-1], out=fu_base[1:])
    fu_base += 2 * nlo_r_pad.sum()
    totpm = int(2 * nlo_r_pad.sum() + nfull_r_pad.sum())

    base_r = np.zeros(NCH, dtype=np.int64)
    np.cumsum(ncols_r[:-1], out=base_r[1:])
    totcols = int(ncols_r.sum())

    ftype = np.float16

    # per-core edge placement
    core_cnt = np.bincount(core_s, minlength=NC)
    core_off = np.zeros(NC + 1, dtype=np.int64)
    np.cumsum(core_cnt, out=core_off[1:])

    per_core = []
    for c in range(NC):
        s, e = core_off[c], core_off[c + 1]
        r_e = chunk_s[s:e]
        g_e = grp_s[s:e]
        gl_e = gl_s[s:e]
        src_e = src_s[s:e]
        ew_e = ew_s[s:e]
        n = e - s
        # rank within slot
        sid = (g_e * NCH + r_e)
        run_starts = np.zeros(n, dtype=np.int64)
        if n:
            brk = np.flatnonzero(np.diff(sid)) + 1
            rb = np.r_[0, brk]
            run_starts = np.repeat(rb, np.diff(np.r_[rb, n]))
        rank = np.arange(n, dtype=np.int64) - run_starts

        nE = cnt[c, g_e, r_e]
        bE = b_c[c, g_e, r_e]
        tE = t_c[c, g_e, r_e]
        kE = k_slot[g_e, r_e]
        fullE = full_slot[g_e, r_e]

        # column-within-slot (= consumption order index) and pos within col
        col_w = np.empty(n, dtype=np.int64)
        pos = np.empty(n, dtype=np.int64)
        is_c0 = (~fullE) & (rank < bE)
        is_cl = (~fullE) & (rank >= nE - tE)
        is_mid = (~fullE) & ~is_c0 & ~is_cl
        col_w[is_c0] = 0
        pos[is_c0] = rank[is_c0]
        col_w[is_cl] = kE[is_cl] - 1
        pos[is_cl] = rank[is_cl] - (nE[is_cl] - tE[is_cl])
        col_w[is_mid] = 1 + (rank[is_mid] - bE[is_mid]) // P
        pos[is_mid] = (rank[is_mid] - bE[is_mid]) % P
        col_w[fullE] = rank[fullE] // P
        pos[fullE] = rank[fullE] % P

        # gather slot (consumption order per chunk)
        gcol = cidx_base[g_e, r_e] + col_w
        gslot = (base_r[r_e] + gcol) * P + pos

        # pm slot (stream-major layout)
        pm_col = np.empty(n, dtype=np.int64)
        dstl_v = np.empty(n, dtype=np.float32)
        pm_col[is_c0] = lo_base[r_e[is_c0]] + lo_idx[g_e[is_c0], r_e[is_c0]]
        dstl_v[is_c0] = gl_e[is_c0]
        pm_col[is_cl] = hi_base[r_e[is_cl]] + lo_idx[g_e[is_cl], r_e[is_cl]]
        dstl_v[is_cl] = gl_e[is_cl] - HI0
        is_f = is_mid | fullE
        off_f = np.where(fullE, col_w, col_w - 1)
        pm_col[is_f] = (fu_base[r_e[is_f]] + full_base[g_e[is_f], r_e[is_f]]
                        + off_f[is_f])
        dstl_v[is_f] = gl_e[is_f]
        pmslot = pm_col * P + pos

        idx_flat = np.zeros(totcols * P, dtype=np.int16)
        idx_flat[gslot] = (src_e - r_e * CHROWS).astype(np.int16)
        dstl_flat = np.full(totpm * P, -1.0, dtype=ftype)
        w_flat = np.zeros(totpm * P, dtype=ftype)
        dstl_flat[pmslot] = dstl_v.astype(ftype)
        w_flat[pmslot] = ew_e.astype(ftype)

        dstl_tab = np.ascontiguousarray(dstl_flat.reshape(totpm, P).T)
        w_tab = np.ascontiguousarray(w_flat.reshape(totpm, P).T)

        # idx16 ucode tables: per chunk, wrapped [16, cols*8] then tiled x8
        idx_parts = []
        for r in range(NCH):
            b0, b1 = base_r[r] * P, (base_r[r] + ncols_r[r]) * P
            seg = idx_flat[b0:b1]
            t16 = seg.reshape(-1, 16).T
            idx_parts.append(np.tile(t16, (8, 1)))
        idx_tab = np.ascontiguousarray(np.concatenate(idx_parts, axis=1))

        xdtype = ml_dtypes.float8_e3m4 if USE_FP8 else ftype
        xdt = np.ascontiguousarray(
            x_dst[c * SHARD:(c + 1) * SHARD].T.astype(xdtype))
        per_core.append({"dstl": dstl_tab, "w": w_tab, "idx16": idx_tab,
                         "xdt": xdt})

    # per-group column metadata for the program (shared across cores)
    # col entry: (chunk, gather col cidx, pm col index, width, psum offset)
    group_cols = []
    for g in range(NG):
        cols = []
        for r in range(NCH):
            k = int(k_slot[g, r])
            if k == 0:
                continue
            cb = int(cidx_base[g, r])
            if full_slot[g, r]:
                for j in range(k):
                    cols.append((r, cb + j,
                                 int(fu_base[r] + full_base[g, r] + j),
                                 G, 0))
            else:
                cols.append((r, cb,
                             int(lo_base[r] + lo_idx[g, r]), WLO, 0))
                for j in range(1, k - 1):
                    cols.append((r, cb + j,
                                 int(fu_base[r] + full_base[g, r] + j - 1),
                                 G, 0))
                cols.append((r, cb + k - 1,
                             int(hi_base[r] + lo_idx[g, r]), WLO, HI0))
        group_cols.append(cols)

    pm_bounds = sorted(set(
        [int(b) // KB for b in lo_base] + [int(b) // KB for b in hi_base]
        + [int(b) // KB for b in fu_base] + [totpm // KB]))
    meta = {
        "ncols_r": ncols_r, "base_r": base_r, "totcols": totcols,
        "totpm": totpm, "group_cols": group_cols,
        "lo_hi_end": int(2 * nlo_r_pad.sum()),
        "pm_bounds": pm_bounds,
    }
    common = {
        "iota48": np.tile(np.repeat(np.arange(WLO), KB).astype(ftype),
                          (P, 1)),
        "iota70": np.tile(np.repeat(np.arange(G), KB).astype(ftype), (P, 1)),
        "wn": np.ascontiguousarray(np.asarray(W_nei, np.float32).T
                                   .astype(ftype)),
        "ws": np.ascontiguousarray(np.asarray(W_self, np.float32).T
                                   .astype(ftype)),
        "bias": np.asarray(b_self, np.float32).reshape(1, F).astype(ftype),
        "ones": np.ones((1, TCH), ftype),
        "zeros": np.zeros((1, F), ftype),
    }
    return meta, per_core, common


def _build_program(meta):
    ncols_r = meta["ncols_r"]
    base_r = meta["base_r"]
    totcols = meta["totcols"]
    totpm = meta["totpm"]
    group_cols = meta["group_cols"]
    lo_hi_end = meta["lo_hi_end"]
    totidx = totcols * 8

    nc = bacc.Bacc("TRN2", target_bir_lowering=False, debug=False,
                   enable_asserts=False, num_devices=NC,
                   dynamic_dma_scratch_size=DMA_SCRATCH)
    f32 = mybir.dt.float32
    DT = mybir.dt.float16
    DT8 = mybir.dt.float8e3 if USE_FP8 else DT
    xcols = XPAD8 if USE_FP8 else XPAD
    x_src_t = nc.dram_tensor("x_src", (N_SRC, xcols), DT8,
                             kind="ExternalInput")
    xdt_t = nc.dram_tensor("xdt", (F, SHARD), DT8, kind="ExternalInput")
    idx_t = nc.dram_tensor("idx16", (P, totidx), mybir.dt.int16,
                           kind="ExternalInput")
    dstl_t = nc.dram_tensor("dstl", (P, totpm), DT, kind="ExternalInput")
    w_t = nc.dram_tensor("w", (P, totpm), DT, kind="ExternalInput")
    iota48_t = nc.dram_tensor("iota48", (P, WLO * KB), DT,
                              kind="ExternalInput")
    iota70_t = nc.dram_tensor("iota70", (P, G * KB), DT,
                              kind="ExternalInput")
    wn_t = nc.dram_tensor("wn", (F, F), DT, kind="ExternalInput")
    ws_t = nc.dram_tensor("ws", (F, F), DT, kind="ExternalInput")
    bias_t = nc.dram_tensor("bias", (1, F), DT, kind="ExternalInput")
    ones_t = nc.dram_tensor("ones", (1, TCH), DT, kind="ExternalInput")
    zeros_t = nc.dram_tensor("zeros", (1, F), DT, kind="ExternalInput")
    out_t = nc.dram_tensor("outT", (F, SHARD), DT, kind="ExternalOutput")

    gather_fn = _get_patched_gather(nc)

    # variable gather window widths per chunk: ramp-in, W steady, modest tail
    def mk_widths(cr):
        widths = []
        rem = int(cr)
        for w0 in (16, 32):
            if rem <= 0:
                break
            take = min(w0, rem)
            widths.append(take)
            rem -= take
        while rem > 0:
            take = min(W, rem)
            if take == W and 0 < rem - take < 32:
                take = rem - 32
            widths.append(take)
            rem -= take
        return widths

    win_widths = [mk_widths(ncols_r[r]) for r in range(NCH)]
    win_starts = []
    for r in range(NCH):
        st, acc = [], 0
        for w0 in win_widths[r]:
            st.append(acc)
            acc += w0
        win_starts.append(st)
    n_win = max(len(ws_) for ws_ in win_widths)

    def col_to_win(r, o):
        import bisect
        return bisect.bisect_right(win_starts[r], o) - 1

    # per-group: max gather window needed
    gwin = []
    n_bat = totpm // KB
    for g in range(NG):
        wk = 0
        for (r, cidx, pmc, wd, off) in group_cols[g]:
            wk = max(wk, col_to_win(r, cidx))
        gwin.append(wk)

    with tile.TileContext(nc) as tc:
        with ExitStack() as ctx:
            const = ctx.enter_context(tc.tile_pool(name="const", bufs=1))
            msgp = [ctx.enter_context(tc.tile_pool(name=f"msg{r}", bufs=2))
                    for r in range(NCH)]
            megs = ctx.enter_context(tc.tile_pool(name="megs", bufs=4))
            megp48 = ctx.enter_context(tc.tile_pool(name="mega48", bufs=20))
            megp70 = ctx.enter_context(tc.tile_pool(name="mega70", bufs=8))
            aggp = ctx.enter_context(tc.tile_pool(name="agg", bufs=3))
            xdtp = ctx.enter_context(tc.tile_pool(name="xdtp", bufs=3))
            outp = ctx.enter_context(tc.tile_pool(name="outp", bufs=3))
            psg = ctx.enter_context(tc.tile_pool(name="psg", bufs=6,
                                                 space="PSUM"))
            pst = ctx.enter_context(tc.tile_pool(name="pst", bufs=2,
                                                 space="PSUM"))

            idx_rs = []
            for r in range(NCH):
                i0 = int(base_r[r]) * 8
                i1 = i0 + int(ncols_r[r]) * 8
                idx_r = const.tile([P, i1 - i0], mybir.dt.int16,
                                   tag=f"idx{r}")
                nc.sync.dma_start(idx_r[:], idx_t.ap()[:, i0:i1])
                idx_rs.append(idx_r)
            iota48_s = const.tile([P, WLO * KB], DT)
            nc.sync.dma_start(iota48_s[:], iota48_t.ap())
            iota70_s = const.tile([P, G * KB], DT)
            nc.sync.dma_start(iota70_s[:], iota70_t.ap())
            dstl_s = const.tile([P, totpm], DT)
            nc.sync.dma_start(dstl_s[:], dstl_t.ap())
            w_s = const.tile([P, totpm], DT)
            nc.sync.dma_start(w_s[:], w_t.ap())
            wn_s = const.tile([F, F], DT)
            nc.sync.dma_start(wn_s[:], wn_t.ap())
            ws_s = const.tile([F, F], DT)
            nc.sync.dma_start(ws_s[:], ws_t.ap())
            bias_s = const.tile([1, F], DT)
            nc.sync.dma_start(bias_s[:], bias_t.ap())
            ones_s = const.tile([1, TCH], DT)
            nc.sync.dma_start(ones_s[:], ones_t.ap())
            zeros_s = const.tile([1, F], DT)
            nc.sync.dma_start(zeros_s[:], zeros_t.ap())

            win_tiles = [[None] * n_win for _ in range(NCH)]
            bat_tiles = [None] * n_bat

            def emit_batch(bk):
                tb0 = bk * KB
                wd = WLO if tb0 < lo_hi_end else G
                iota = iota48_s if wd == WLO else iota70_s
                eq = megs.tile([P, G * KB], DT, tag="eq")
                nc.vector.tensor_tensor(
                    out=eq[:, :wd * KB].rearrange("p (g k) -> p g k", k=KB),
                    in0=iota[:].rearrange("p (g k) -> p g k", k=KB),
                    in1=dstl_s[:, tb0:tb0 + KB].unsqueeze(1)
                        .broadcast_to([P, wd, KB]),
                    op=mybir.AluOpType.is_equal)
                pool = megp48 if wd == WLO else megp70
                pm = pool.tile([P, wd * KB], DT, tag="pm")
                nc.vector.tensor_tensor(
                    out=pm[:].rearrange("p (g k) -> p g k", k=KB),
                    in0=eq[:, :wd * KB].rearrange("p (g k) -> p g k", k=KB),
                    in1=w_s[:, tb0:tb0 + KB].unsqueeze(1)
                        .broadcast_to([P, wd, KB]),
                    op=mybir.AluOpType.mult)
                bat_tiles[bk] = (pm, wd)

            def emit_window(wk):
                for r in range(NCH):
                    if wk >= len(win_widths[r]):
                        continue
                    c0 = win_starts[r][wk]
                    wcols = int(win_widths[r][wk])
                    mt = msgp[r].tile([P, W * F], DT8, tag=f"m{r}")
                    out3d = mt[:, :wcols * F].rearrange(
                        "p (c f) -> p c f", f=F)
                    i0 = c0 * 8
                    nidx = wcols * P
                    gather_fn(
                        nc.gpsimd,
                        out_ap=out3d,
                        in_ap=x_src_t.ap()[r * CHROWS:(r + 1) * CHROWS, :F],
                        idxs_ap=idx_rs[r][:, i0:i0 + wcols * 8],
                        num_idxs=nidx, num_idxs_reg=nidx, elem_size=F,
                        elem_step=XPAD8 if USE_FP8 else XPAD,
                        single_packet=False)
                    win_tiles[r][wk] = mt

            import bisect as _bisect
            pm_bounds = meta["pm_bounds"]
            sub_ptr = {b: b for b in pm_bounds[:-1]}

            def ensure_batch(bk):
                s = pm_bounds[_bisect.bisect_right(pm_bounds, bk) - 1]
                while sub_ptr[s] <= bk:
                    emit_batch(sub_ptr[s])
                    sub_ptr[s] += 1

            emitted = 0
            for t in range(NTC):
                csize = min(TCH, SHARD - t * TCH)
                glo = t * (TCH // G)
                ghi = min(NG, glo + (TCH // G))
                agg_tile = aggp.tile([F, TCH], DT, tag="agg")
                for g in range(glo, ghi):
                    while emitted <= gwin[g] and emitted < n_win:
                        emit_window(emitted)
                        emitted += 1
                    for (r, cidx, pmc, wd, off) in group_cols[g]:
                        ensure_batch(pmc // KB)
                    gsize = min(G, SHARD - g * G)
                    ps = psg.tile([F, G], f32, tag="ps")
                    cols = group_cols[g]
                    nc.tensor.matmul(
                        out=ps[:], lhsT=zeros_s[:], rhs=ones_s[:, :G],
                        start=True, stop=(len(cols) == 0))
                    for j, (r, cidx, pmc, wd, off) in enumerate(cols):
                        o = cidx
                        lcw = col_to_win(r, o)
                        lc = o - win_starts[r][lcw]
                        mt = win_tiles[r][lcw]
                        pm, bwd = bat_tiles[pmc // KB]
                        jk = pmc % KB
                        rhs = pm[:, :bwd * KB].rearrange(
                            "p (g k) -> p g k", k=KB)[:, :, jk]
                        nc.tensor.matmul(
                            out=ps[:, off:off + wd],
                            lhsT=mt[:, lc * F:(lc + 1) * F],
                            rhs=rhs, start=False,
                            stop=(j == len(cols) - 1))
                    offb = (g - glo) * G
                    nc.scalar.copy(agg_tile[:, offb:offb + gsize],
                                   ps[:, :gsize])
                # transform this chunk of TCH dsts
                xdt_s = xdtp.tile([F, TCH], DT8, tag="xdt")
                nc.sync.dma_start(xdt_s[:, :csize],
                                  xdt_t.ap()[:, t * TCH:t * TCH + csize])
                ps2 = pst.tile([F, TCH], f32, tag="ps2")
                nc.tensor.matmul(out=ps2[:, :csize], lhsT=wn_s[:],
                                 rhs=agg_tile[:, :csize], start=True,
                                 stop=False)
                nc.tensor.matmul(out=ps2[:, :csize], lhsT=bias_s[:],
                                 rhs=ones_s[:, :csize], start=False,
                                 stop=False)
                nc.tensor.matmul(out=ps2[:, :csize], lhsT=ws_s[:],
                                 rhs=xdt_s[:, :csize], start=False, stop=True)
                osb = outp.tile([F, TCH], DT, tag="osb")
                nc.scalar.copy(osb[:, :csize], ps2[:, :csize])
                nc.sync.dma_start(out_t.ap()[:, t * TCH:t * TCH + csize],
                                  osb[:, :csize])

    nc.compile()
    return nc


def _prep_x_src(x_src):
    x_src = np.asarray(x_src, dtype=np.float32)
    if USE_FP8:
        xp = np.zeros((N_SRC, XPAD8), dtype=ml_dtypes.float8_e3m4)
        xp[:, :F] = x_src.astype(ml_dtypes.float8_e3m4)
        return xp
    xp = np.zeros((N_SRC, XPAD), dtype=np.float16)
    xp[:, :F] = x_src.astype(np.float16)
    return xp


def run(inputs, trace=False):
    meta, per_core, common = _host_prep(
        inputs["x_src"], inputs["x_dst"], inputs["edge_index_sd"],
        inputs["edge_weight"], inputs["W_nei"], inputs["W_self"],
        inputs["b_self"])
    nc = _build_program(meta)
    xs = _prep_x_src(inputs["x_src"])
    in_maps = []
    for c in range(NC):
        m = {"x_src": xs}
        m.update(common)
        m.update(per_core[c])
        in_maps.append(m)
    res = run_bass_kernel_spmd(nc, in_maps, core_ids=list(range(NC)),
                               trace=trace)
    out = np.empty((N_DST, F), dtype=np.float32)
    for c in range(NC):
        out[c * SHARD:(c + 1) * SHARD] = \
            res.results[c]["outT"].T.astype(np.float32)
    return out, res


def kernel(**inputs) -> np.ndarray:
    out, _ = run(inputs, trace=False)
    return out


# revision 78
# speedup vs baseline: 1.2031x; 1.2031x over previous
"""BiGraphConv (GNN message passing) Trainium2 kernel, 8-core SPMD.

out = x_dst @ W_self.T + b_self + scatter_add_dst(w_e * x_src[src_e]) @ W_nei.T

Formulated aggregate-first, per dst-shard:
    agg[d]  = sum_{e: dst_e=d} w_e * x_src[src_e]     (gather + one-hot matmul)
    out'[d] = W_nei @ agg[d] + W_self @ x_dst[d] + b  (feature-major matmuls)

Sharding: dst nodes partitioned across 8 cores (12500 each); x_src replicated;
edges bucketed by (dst-core, dst-group, src-chunk) on host. Gathered x rows in
fp8 e3m4 (64B descriptors at the DMA cost floor); one-hot construction in fp16
with narrowed sub-windows per column rank (col0 covers group-local dst
[0,48), col-last [22,70), middles full 70) to cut DVE work; fp32 PSUM
accumulation with groups paired per PSUM tile (one zeros-init matmul + one
ACT copy per pair); two-matmul fp16 transform (W_nei@agg + W_self@x_dst) with
b_self folded into the ACT output downcast; f16 output upcast on host.
"""
import sys
import inspect
import re
import numpy as np

for _p in ("/opt/trn_rl_repo", "/root/.axon_site/_ro/trn_rl_repo"):
    if _p not in sys.path:
        sys.path.insert(0, _p)

from contextlib import ExitStack

import ml_dtypes
import concourse.bass as bass
import concourse.tile as tile
from concourse import bacc, mybir
from concourse.bass_utils import run_bass_kernel_spmd

# problem constants (hardcoded per task contract)
N_SRC = 100000
N_DST = 100000
E = 1250000
F = 64          # feature dim (in == out == 64)
NC = 8          # cores
SHARD = N_DST // NC          # 12500 dst rows per core
G = 70                       # dst rows per aggregation group
NG = (SHARD + G - 1) // G    # 179 groups per core
NCH = 4                      # src chunks (int16 index limit)
CHROWS = N_SRC // NCH        # 25000 rows per chunk window
W = 96                       # gather window width in 128-edge columns
                             # (ring limit: W*128/16+1 <= 1024 descriptors)
KB = 16                      # one-hot batch width in columns
DMA_SCRATCH = 16384          # SWDGE ring bytes per partition (default)
TCH = 490                    # transform chunk (dst cols; multiple of G)
NTC = (SHARD + TCH - 1) // TCH   # 26 transform chunks
USE_FP8 = True               # fp8 e3m4 gathered x rows + fp8 x_dst

# narrowed one-hot sub-windows (group-local dst ranges per column rank)
WLO = 44                     # col0 window [0, WLO)
HI0 = G - WLO                # col-last window [HI0, G) = [22, 70)
IDXSPLIT = 48                # idx table split point (cols) for early gathers

P = 128
XPAD = 128                   # padded fp16 row length (256B stride)
XPAD8 = 256                  # padded fp8 row length (256B stride)

_patched_gather = None


def _get_patched_gather(nc):
    """dma_gather with the 256B-payload assert relaxed for non-transpose.

    The ucode's row-stride field is in 256B units (elem_step stays 256B via
    the padded source), but the payload may be 64/128B; verified on HW.
    """
    global _patched_gather
    if _patched_gather is not None:
        return _patched_gather
    cls = type(nc.gpsimd)
    src = inspect.getsource(cls.dma_gather)
    src = src.replace(
        """        assert (
            elem_size_bytes > 0 and elem_size_bytes % 256 == 0
        )  # transpose restriction""",
        """        assert elem_size_bytes > 0
        if transpose:
            assert elem_size_bytes % 256 == 0""")
    src = re.sub(r"^    def dma_gather", "def dma_gather", src)
    src = re.sub(r"\n    ", "\n", src)
    ns = vars(sys.modules[cls.__module__]).copy()
    exec(compile(src, "<patched_dma_gather>", "exec"), ns)
    _patched_gather = ns["dma_gather"]
    return _patched_gather


def _host_prep(x_src, x_dst, edge_index_sd, edge_weight, W_nei, W_self, b_self):
    src = np.asarray(edge_index_sd[0], dtype=np.int64)
    dst = np.asarray(edge_index_sd[1], dtype=np.int64)
    ew = np.asarray(edge_weight, dtype=np.float32)
    x_dst = np.asarray(x_dst, dtype=np.float32)

    core = dst // SHARD
    dl = dst % SHARD          # shard-local dst id
    grp = dl // G
    gl = dl % G               # group-local dst id
    chunk = src // CHROWS

    order = np.lexsort((gl, chunk, grp, core))
    core_s = core[order]
    chunk_s = chunk[order]
    grp_s = grp[order]
    gl_s = gl[order]
    src_s = src[order]
    ew_s = ew[order]

    # per (core, group, chunk) slot counts + sub-window counts
    key = (core_s * NG + grp_s) * NCH + chunk_s
    NSLOT = NC * NG * NCH
    cnt = np.bincount(key, minlength=NSLOT).reshape(NC, NG, NCH)
    nb_lo = np.bincount(key[gl_s < HI0], minlength=NSLOT).reshape(
        NC, NG, NCH)                       # edges that MUST go to col0
    nb_hi = np.bincount(key[gl_s >= WLO], minlength=NSLOT).reshape(
        NC, NG, NCH)                       # edges that MUST go to col-last

    # shared structure: columns per slot = max over cores
    k_slot = (-(-cnt // P)).max(axis=0)    # [NG, NCH]
    # per-core assignment: top t -> col-last, bottom b -> col0, rest middles
    t_c = np.minimum(P, cnt - nb_lo)       # col-last load (all gl >= HI0..)
    t_c = np.maximum(t_c, 0)
    b_c = np.minimum(np.minimum(P, cnt - nb_hi), cnt - t_c)
    b_c = np.maximum(b_c, 0)
    m_c = cnt - t_c - b_c
    # fallback: full-width columns when narrowed windows infeasible anywhere
    full_slot = (k_slot == 1) | (m_c > P * np.maximum(k_slot - 2, 0)).any(
        axis=0)
    full_slot &= k_slot > 0

    # column inventory per chunk, group-ascending (= gather/consumption order)
    # within a slot: [col0(lo), middles(full), col-last(hi)] or all-full
    ncols_slot = k_slot                      # [NG, NCH]
    cidx_base = np.zeros((NG, NCH), dtype=np.int64)
    for r in range(NCH):
        np.cumsum(ncols_slot[:-1, r], out=cidx_base[1:, r])
    ncols_r = ncols_slot.sum(axis=0)         # gather cols per chunk

    # pm stream indices
    is_lo = (~full_slot) & (k_slot >= 2)     # slots contributing lo+hi cols
    lo_idx = np.zeros((NG, NCH), dtype=np.int64)
    nfull_slot = np.where(full_slot, k_slot, np.maximum(k_slot - 2, 0))
    full_base = np.zeros((NG, NCH), dtype=np.int64)
    nlo_r = np.zeros(NCH, dtype=np.int64)
    nfull_r = np.zeros(NCH, dtype=np.int64)
    for r in range(NCH):
        lo_idx[:, r] = np.cumsum(is_lo[:, r]) - is_lo[:, r]
        nlo_r[r] = is_lo[:, r].sum()
        full_base[:, r] = np.cumsum(nfull_slot[:, r]) - nfull_slot[:, r]
        nfull_r[r] = nfull_slot[:, r].sum()

    def pad16(n):
        return -(-int(n) // KB) * KB

    nlo_r_pad = np.array([pad16(n) for n in nlo_r])
    nfull_r_pad = np.array([pad16(n) for n in nfull_r])
    # pm table layout: [lo r0..r3 | hi r0..r3 | full r0..r3]
    lo_base = np.zeros(NCH, dtype=np.int64)
    np.cumsum(nlo_r_pad[:-1], out=lo_base[1:])
    hi_base = np.zeros(NCH, dtype=np.int64)
    np.cumsum(nlo_r_pad[:-1], out=hi_base[1:])
    hi_base += nlo_r_pad.sum()
    fu_base = np.zeros(NCH, dtype=np.int64)
    np.cumsum(nfull_r_pad[:-1], out=fu_base[1:])
    fu_base += 2 * nlo_r_pad.sum()
    lo_hi_end = int(2 * nlo_r_pad.sum())
    totpm = lo_hi_end + int(nfull_r_pad.sum())

    base_r = np.zeros(NCH, dtype=np.int64)
    np.cumsum(ncols_r[:-1], out=base_r[1:])
    totcols = int(ncols_r.sum())

    ftype = np.float16

    # per-core edge placement
    core_cnt = np.bincount(core_s, minlength=NC)
    core_off = np.zeros(NC + 1, dtype=np.int64)
    np.cumsum(core_cnt, out=core_off[1:])

    per_core = []
    for c in range(NC):
        s, e = core_off[c], core_off[c + 1]
        r_e = chunk_s[s:e]
        g_e = grp_s[s:e]
        gl_e = gl_s[s:e]
        src_e = src_s[s:e]
        ew_e = ew_s[s:e]
        n = e - s
        # rank within slot
        sid = (g_e * NCH + r_e)
        run_starts = np.zeros(n, dtype=np.int64)
        if n:
            brk = np.flatnonzero(np.diff(sid)) + 1
            rb = np.r_[0, brk]
            run_starts = np.repeat(rb, np.diff(np.r_[rb, n]))
        rank = np.arange(n, dtype=np.int64) - run_starts

        nE = cnt[c, g_e, r_e]
        bE = b_c[c, g_e, r_e]
        tE = t_c[c, g_e, r_e]
        kE = k_slot[g_e, r_e]
        fullE = full_slot[g_e, r_e]

        # column-within-slot (= consumption order index) and pos within col
        col_w = np.empty(n, dtype=np.int64)
        pos = np.empty(n, dtype=np.int64)
        is_c0 = (~fullE) & (rank < bE)
        is_cl = (~fullE) & (rank >= nE - tE)
        is_mid = (~fullE) & ~is_c0 & ~is_cl
        col_w[is_c0] = 0
        pos[is_c0] = rank[is_c0]
        col_w[is_cl] = kE[is_cl] - 1
        pos[is_cl] = rank[is_cl] - (nE[is_cl] - tE[is_cl])
        col_w[is_mid] = 1 + (rank[is_mid] - bE[is_mid]) // P
        pos[is_mid] = (rank[is_mid] - bE[is_mid]) % P
        col_w[fullE] = rank[fullE] // P
        pos[fullE] = rank[fullE] % P

        # gather slot (consumption order per chunk)
        gcol = cidx_base[g_e, r_e] + col_w
        gslot = (base_r[r_e] + gcol) * P + pos

        # pm slot (stream-major layout)
        pm_col = np.empty(n, dtype=np.int64)
        dstl_v = np.empty(n, dtype=np.float32)
        pm_col[is_c0] = lo_base[r_e[is_c0]] + lo_idx[g_e[is_c0], r_e[is_c0]]
        dstl_v[is_c0] = gl_e[is_c0]
        pm_col[is_cl] = hi_base[r_e[is_cl]] + lo_idx[g_e[is_cl], r_e[is_cl]]
        dstl_v[is_cl] = gl_e[is_cl] - HI0
        is_f = is_mid | fullE
        off_f = np.where(fullE, col_w, col_w - 1)
        pm_col[is_f] = (fu_base[r_e[is_f]] + full_base[g_e[is_f], r_e[is_f]]
                        + off_f[is_f])
        dstl_v[is_f] = gl_e[is_f]
        pmslot = pm_col * P + pos

        idx_flat = np.zeros(totcols * P, dtype=np.int16)
        idx_flat[gslot] = (src_e - r_e * CHROWS).astype(np.int16)
        dstl_flat = np.full(totpm * P, -1.0, dtype=ftype)
        w_flat = np.zeros(totpm * P, dtype=ftype)
        dstl_flat[pmslot] = dstl_v.astype(ftype)
        w_flat[pmslot] = ew_e.astype(ftype)

        dstl_tab = np.ascontiguousarray(dstl_flat.reshape(totpm, P).T)
        w_tab = np.ascontiguousarray(w_flat.reshape(totpm, P).T)

        # idx16 ucode tables: per chunk, wrapped [16, cols*8] then tiled x8
        idx_parts = []
        for r in range(NCH):
            b0, b1 = base_r[r] * P, (base_r[r] + ncols_r[r]) * P
            seg = idx_flat[b0:b1]
            t16 = seg.reshape(-1, 16).T
            idx_parts.append(np.tile(t16, (8, 1)))
        idx_tab = np.ascontiguousarray(np.concatenate(idx_parts, axis=1))

        xdtype = ml_dtypes.float8_e3m4 if USE_FP8 else ftype
        xdt = np.ascontiguousarray(
            x_dst[c * SHARD:(c + 1) * SHARD].T.astype(xdtype))
        per_core.append({"dstl": dstl_tab, "w": w_tab, "idx16": idx_tab,
                         "xdt": xdt})

    # per-group column metadata for the program (shared across cores)
    # col entry: (chunk, gather col cidx, pm col index, width, psum offset)
    group_cols = []
    for g in range(NG):
        cols = []
        for r in range(NCH):
            k = int(k_slot[g, r])
            if k == 0:
                continue
            cb = int(cidx_base[g, r])
            if full_slot[g, r]:
                for j in range(k):
                    cols.append((r, cb + j,
                                 int(fu_base[r] + full_base[g, r] + j),
                                 G, 0))
            else:
                cols.append((r, cb,
                             int(lo_base[r] + lo_idx[g, r]), WLO, 0))
                for j in range(1, k - 1):
                    cols.append((r, cb + j,
                                 int(fu_base[r] + full_base[g, r] + j - 1),
                                 G, 0))
                cols.append((r, cb + k - 1,
                             int(hi_base[r] + lo_idx[g, r]), WLO, HI0))
        group_cols.append(cols)

    pm_bounds = sorted(set(
        [int(b) // KB for b in lo_base] + [int(b) // KB for b in hi_base]
        + [int(b) // KB for b in fu_base] + [totpm // KB]))
    meta = {
        "ncols_r": ncols_r, "base_r": base_r, "totcols": totcols,
        "totpm": totpm, "group_cols": group_cols,
        "lo_hi_end": lo_hi_end,
        "pm_bounds": pm_bounds,
    }
    common = {
        "iota48": np.tile(np.repeat(np.arange(WLO), KB).astype(ftype),
                          (P, 1)),
        "iota70": np.tile(np.repeat(np.arange(G), KB).astype(ftype), (P, 1)),
        "wn": np.ascontiguousarray(np.asarray(W_nei, np.float32).T
                                   .astype(ftype)),
        "ws": np.ascontiguousarray(np.asarray(W_self, np.float32).T
                                   .astype(ftype)),
        "bias": np.asarray(b_self, np.float32).reshape(F, 1).astype(ftype),
        "ones": np.ones((1, TCH), ftype),
        "zeros": np.zeros((1, F), ftype),
    }
    return meta, per_core, common


def _build_program(meta):
    ncols_r = meta["ncols_r"]
    base_r = meta["base_r"]
    totcols = meta["totcols"]
    totpm = meta["totpm"]
    group_cols = meta["group_cols"]
    lo_hi_end = meta["lo_hi_end"]
    totidx = totcols * 8

    nc = bacc.Bacc("TRN2", target_bir_lowering=False, debug=False,
                   enable_asserts=False, num_devices=NC,
                   dynamic_dma_scratch_size=DMA_SCRATCH)
    f32 = mybir.dt.float32
    DT = mybir.dt.float16
    DT8 = mybir.dt.float8e3 if USE_FP8 else DT
    xcols = XPAD8 if USE_FP8 else XPAD
    x_src_t = nc.dram_tensor("x_src", (N_SRC, xcols), DT8,
                             kind="ExternalInput")
    xdt_t = nc.dram_tensor("xdt", (F, SHARD), DT8, kind="ExternalInput")
    idx_t = nc.dram_tensor("idx16", (P, totidx), mybir.dt.int16,
                           kind="ExternalInput")
    dstl_t = nc.dram_tensor("dstl", (P, totpm), DT, kind="ExternalInput")
    w_t = nc.dram_tensor("w", (P, totpm), DT, kind="ExternalInput")
    iota48_t = nc.dram_tensor("iota48", (P, WLO * KB), DT,
                              kind="ExternalInput")
    iota70_t = nc.dram_tensor("iota70", (P, G * KB), DT,
                              kind="ExternalInput")
    wn_t = nc.dram_tensor("wn", (F, F), DT, kind="ExternalInput")
    ws_t = nc.dram_tensor("ws", (F, F), DT, kind="ExternalInput")
    bias_t = nc.dram_tensor("bias", (F, 1), DT, kind="ExternalInput")
    ones_t = nc.dram_tensor("ones", (1, TCH), DT, kind="ExternalInput")
    zeros_t = nc.dram_tensor("zeros", (1, F), DT, kind="ExternalInput")
    out_t = nc.dram_tensor("outT", (F, SHARD), DT, kind="ExternalOutput")

    gather_fn = _get_patched_gather(nc)

    # variable gather window widths per chunk: ramp-in, W steady, modest tail
    def mk_widths(cr):
        widths = []
        rem = int(cr)
        for w0 in (16, 32):
            if rem <= 0:
                break
            take = min(w0, rem)
            widths.append(take)
            rem -= take
        while rem > 0:
            take = min(W, rem)
            if take == W and 0 < rem - take < 32:
                take = rem - 32
            widths.append(take)
            rem -= take
        return widths

    win_widths = [mk_widths(ncols_r[r]) for r in range(NCH)]
    win_starts = []
    for r in range(NCH):
        st, acc = [], 0
        for w0 in win_widths[r]:
            st.append(acc)
            acc += w0
        win_starts.append(st)
    n_win = max(len(ws_) for ws_ in win_widths)

    def col_to_win(r, o):
        import bisect
        return bisect.bisect_right(win_starts[r], o) - 1

    # per-group, per-chunk: max gather window needed
    gwin = []
    n_bat = totpm // KB
    for g in range(NG):
        wk = [-1] * NCH
        for (r, cidx, pmc, wd, off) in group_cols[g]:
            wk[r] = max(wk[r], col_to_win(r, cidx))
        gwin.append(wk)

    with tile.TileContext(nc) as tc:
        with ExitStack() as ctx:
            const = ctx.enter_context(tc.tile_pool(name="const", bufs=1))
            msgp = [ctx.enter_context(tc.tile_pool(name=f"msg{r}", bufs=2))
                    for r in range(NCH)]
            megs = ctx.enter_context(tc.tile_pool(name="megs", bufs=4))
            megp48 = ctx.enter_context(tc.tile_pool(name="mega48", bufs=20))
            megp70 = ctx.enter_context(tc.tile_pool(name="mega70", bufs=8))
            aggp = ctx.enter_context(tc.tile_pool(name="agg", bufs=3))
            outp = ctx.enter_context(tc.tile_pool(name="outp", bufs=3))
            psg = ctx.enter_context(tc.tile_pool(name="psg", bufs=6,
                                                 space="PSUM"))
            pst = ctx.enter_context(tc.tile_pool(name="pst", bufs=2,
                                                 space="PSUM"))

            # dstl/w ahead of idx: the DVE one-hot stream is the longest
            # dependency chain, so its tables must land first.
            iota48_s = const.tile([P, WLO * KB], DT)
            nc.sync.dma_start(iota48_s[:], iota48_t.ap())
            iota70_s = const.tile([P, G * KB], DT)
            nc.sync.dma_start(iota70_s[:], iota70_t.ap())
            dstl_s = const.tile([P, totpm], DT)
            nc.sync.dma_start(dstl_s[:], dstl_t.ap())
            w_s = const.tile([P, totpm], DT)
            nc.sync.dma_start(w_s[:], w_t.ap())
            idx_rs = []
            for r in range(NCH):
                i0 = int(base_r[r]) * 8
                i1 = i0 + int(ncols_r[r]) * 8
                isplit = min(i0 + IDXSPLIT * 8, i1)
                idx_a = const.tile([P, isplit - i0], mybir.dt.int16,
                                   tag=f"idxa{r}")
                nc.sync.dma_start(idx_a[:], idx_t.ap()[:, i0:isplit])
                idx_b = None
                if isplit < i1:
                    idx_b = const.tile([P, i1 - isplit], mybir.dt.int16,
                                       tag=f"idxb{r}")
                    nc.sync.dma_start(idx_b[:], idx_t.ap()[:, isplit:i1])
                idx_rs.append([idx_a, idx_b, i1 - isplit])
            wn_s = const.tile([F, F], DT)
            nc.sync.dma_start(wn_s[:], wn_t.ap())
            ws_s = const.tile([F, F], DT)
            nc.sync.dma_start(ws_s[:], ws_t.ap())
            bias_s = const.tile([F, 1], DT)
            nc.sync.dma_start(bias_s[:], bias_t.ap())
            ones_s = const.tile([1, TCH], DT)
            nc.sync.dma_start(ones_s[:], ones_t.ap())
            zeros_s = const.tile([1, F], DT)
            nc.sync.dma_start(zeros_s[:], zeros_t.ap())
            xdt_s = const.tile([F, SHARD], DT8, tag="xdtall")
            nc.sync.dma_start(xdt_s[:], xdt_t.ap())

            def pm_slice(which, tb0):
                tile_ = dstl_s if which == 0 else w_s
                return tile_[:, tb0:tb0 + KB]

            win_tiles = [[None] * n_win for _ in range(NCH)]
            bat_tiles = [None] * n_bat

            def emit_batch(bk):
                tb0 = bk * KB
                wd = WLO if tb0 < lo_hi_end else G
                iota = iota48_s if wd == WLO else iota70_s
                eq = megs.tile([P, G * KB], DT, tag="eq")
                nc.vector.tensor_tensor(
                    out=eq[:, :wd * KB].rearrange("p (g k) -> p g k", k=KB),
                    in0=iota[:].rearrange("p (g k) -> p g k", k=KB),
                    in1=pm_slice(0, tb0).unsqueeze(1)
                        .broadcast_to([P, wd, KB]),
                    op=mybir.AluOpType.is_equal)
                pool = megp48 if wd == WLO else megp70
                pm = pool.tile([P, wd * KB], DT, tag="pm")
                nc.vector.tensor_tensor(
                    out=pm[:].rearrange("p (g k) -> p g k", k=KB),
                    in0=eq[:, :wd * KB].rearrange("p (g k) -> p g k", k=KB),
                    in1=pm_slice(1, tb0).unsqueeze(1)
                        .broadcast_to([P, wd, KB]),
                    op=mybir.AluOpType.mult)
                bat_tiles[bk] = (pm, wd)

            def emit_window(r, wk):
                    c0 = win_starts[r][wk]
                    wcols = int(win_widths[r][wk])
                    mt = msgp[r].tile([P, W * F], DT8, tag=f"m{r}")
                    out3d = mt[:, :wcols * F].rearrange(
                        "p (c f) -> p c f", f=F)
                    idx_a, idx_b, _ = idx_rs[r]
                    if c0 < IDXSPLIT:
                        assert c0 + wcols <= IDXSPLIT
                        idx_ap = idx_a[:, c0 * 8:(c0 + wcols) * 8]
                    else:
                        o0 = (c0 - IDXSPLIT) * 8
                        idx_ap = idx_b[:, o0:o0 + wcols * 8]
                    nidx = wcols * P
                    gather_fn(
                        nc.gpsimd,
                        out_ap=out3d,
                        in_ap=x_src_t.ap()[r * CHROWS:(r + 1) * CHROWS, :F],
                        idxs_ap=idx_ap,
                        num_idxs=nidx, num_idxs_reg=nidx, elem_size=F,
                        elem_step=XPAD8 if USE_FP8 else XPAD,
                        single_packet=False)
                    win_tiles[r][wk] = mt

            import bisect as _bisect
            pm_bounds = meta["pm_bounds"]
            sub_ptr = {b: b for b in pm_bounds[:-1]}

            def ensure_batch(bk):
                s = pm_bounds[_bisect.bisect_right(pm_bounds, bk) - 1]
                while sub_ptr[s] <= bk:
                    emit_batch(sub_ptr[s])
                    sub_ptr[s] += 1

            emitted_r = [0] * NCH
            for t in range(NTC):
                csize = min(TCH, SHARD - t * TCH)
                glo = t * (TCH // G)
                ghi = min(NG, glo + (TCH // G))
                agg_tile = aggp.tile([F, TCH], DT, tag="agg")
                for g0 in range(glo, ghi, 4):
                    gpair = [g for g in range(g0, g0 + 4) if g < ghi]
                    cols = []
                    for i, g in enumerate(gpair):
                        for r in range(NCH):
                            while (emitted_r[r] <= gwin[g][r]
                                   and emitted_r[r] < len(win_widths[r])):
                                emit_window(r, emitted_r[r])
                                emitted_r[r] += 1
                        for (r, cidx, pmc, wd, off) in group_cols[g]:
                            ensure_batch(pmc // KB)
                            cols.append((r, cidx, pmc, wd, off + i * G))
                    gsize = min(len(gpair) * G, SHARD - g0 * G)
                    ps = psg.tile([F, 4 * G], f32, tag="ps")
                    nc.tensor.matmul(
                        out=ps[:, :len(gpair) * G], lhsT=zeros_s[:],
                        rhs=ones_s[:, :len(gpair) * G],
                        start=True, stop=(len(cols) == 0))
                    for j, (r, cidx, pmc, wd, off) in enumerate(cols):
                        o = cidx
                        lcw = col_to_win(r, o)
                        lc = o - win_starts[r][lcw]
                        mt = win_tiles[r][lcw]
                        pm, bwd = bat_tiles[pmc // KB]
                        jk = pmc % KB
                        rhs = pm[:, :bwd * KB].rearrange(
                            "p (g k) -> p g k", k=KB)[:, :, jk]
                        nc.tensor.matmul(
                            out=ps[:, off:off + wd],
                            lhsT=mt[:, lc * F:(lc + 1) * F],
                            rhs=rhs, start=False,
                            stop=(j == len(cols) - 1))
                    offb = (g0 - glo) * G
                    nc.scalar.copy(agg_tile[:, offb:offb + gsize],
                                   ps[:, :gsize])
                # transform this chunk of TCH dsts
                ps2 = pst.tile([F, TCH], f32, tag="ps2")
                nc.tensor.matmul(out=ps2[:, :csize], lhsT=wn_s[:],
                                 rhs=agg_tile[:, :csize], start=True,
                                 stop=False)
                nc.tensor.matmul(out=ps2[:, :csize], lhsT=ws_s[:],
                                 rhs=xdt_s[:, t * TCH:t * TCH + csize],
                                 start=False, stop=True)
                osb = outp.tile([F, TCH], DT, tag="osb")
                nc.scalar.add(osb[:, :csize], ps2[:, :csize], bias_s[:])
                nc.sync.dma_start(out_t.ap()[:, t * TCH:t * TCH + csize],
                                  osb[:, :csize])

    nc.compile()
    return nc


def _prep_x_src(x_src):
    x_src = np.asarray(x_src, dtype=np.float32)
    if USE_FP8:
        xp = np.zeros((N_SRC, XPAD8), dtype=ml_dtypes.float8_e3m4)
        xp[:, :F] = x_src.astype(ml_dtypes.float8_e3m4)
        return xp
    xp = np.zeros((N_SRC, XPAD), dtype=np.float16)
    xp[:, :F] = x_src.astype(np.float16)
    return xp


def run(inputs, trace=False):
    meta, per_core, common = _host_prep(
        inputs["x_src"], inputs["x_dst"], inputs["edge_index_sd"],
        inputs["edge_weight"], inputs["W_nei"], inputs["W_self"],
        inputs["b_self"])
    nc = _build_program(meta)
    xs = _prep_x_src(inputs["x_src"])
    in_maps = []
    for c in range(NC):
        m = {"x_src": xs}
        m.update(common)
        m.update(per_core[c])
        in_maps.append(m)
    res = run_bass_kernel_spmd(nc, in_maps, core_ids=list(range(NC)),
                               trace=trace)
    out = np.empty((N_DST, F), dtype=np.float32)
    for c in range(NC):
        out[c * SHARD:(c + 1) * SHARD] = \
            res.results[c]["outT"].T.astype(np.float32)
    return out, res


def kernel(**inputs) -> np.ndarray:
    out, _ = run(inputs, trace=False)
    return out
